# revision 24
# baseline (speedup 1.0000x reference)
"""Trainium2 Bass kernel for nn_MultiHeadAttn (dense transformer block:
QKV proj -> causal MHA -> out proj -> residual -> LayerNorm).

Sharding: tensor-parallel over the 16 heads across 8 NeuronCores (2 heads
per core). Each core computes Q/K/V for its heads over all tokens, causal
attention with the softmax denominator carried as an appended ones-column
in V, then an AllToAll redistributes the per-head attention vectors so
each core holds all 16 heads for 1/8 of the token rows and applies the
output projection, residual add and LayerNorm for those rows.

The body is structured as a small number of For_i hardware loops with all
varying-operand matmul inputs staged through fixed SBUF addresses (the PE
stationary operand cannot take register offsets), which keeps the static
instruction count low:
  - one QK projection loop over 512-token blocks (weights stationary),
  - one V projection loop over 128-token tiles producing V token-major,
  - four (batch, head) attention loops over the 16 key tiles, computing
    scores for the full query range and masking via a sliding causal-mask
    window, accumulating AV in PSUM across iterations,
  - one output-projection + LayerNorm loop over 128-row output tiles.
"""

import os
import sys

import numpy as np

try:
    import concourse.bass as bass  # noqa: F401
except ImportError:  # pragma: no cover
    sys.path.insert(0, "/opt/trn_rl_repo")

import ml_dtypes

import concourse.bass as bass
from concourse.bass import ds, ts
import concourse.mybir as mybir
import concourse.tile as tile
from concourse import bacc
from concourse.bass_utils import run_bass_kernel_spmd
from concourse.masks import make_upper_triangular

# Problem constants
T_FULL = 2048
B = 2
D_MODEL = 1024
N_HEAD = 16
D_HEAD = 64
LN_EPS = 1e-5
N_CORES = 8
SCALE = 1.0 / (D_HEAD**0.5)
EXP_BIAS = -3.0  # scores are in [-3.3, 3.3] for this problem; keeps exp <= ~1.4

P = 128
KCH = D_MODEL // P  # 8 contraction chunks

F32 = mybir.dt.float32
BF16 = mybir.dt.bfloat16

# Stash of the most recent run's BassKernelResults (for test harnesses).
LAST_RESULT = None


def build_program(t=T_FULL, n_cores=N_CORES, repeat=1, no_collective=False, apply_gb=True,
                  debug=False):
    """Builds the SPMD Bass program (same program on every core).

    repeat > 1 re-emits the whole kernel body (everything except constant
    weight loads) that many times — used only for wall-clock timing.
    """
    nh_loc = N_HEAD // n_cores  # 2 heads per core
    assert nh_loc == 2
    bt = B * t  # flattened (batch, token) axis, batch-major
    cs = t // n_cores  # per-batch token chunk per core (A2A shard)
    n_it = B * cs // P  # 128-row output tiles per core (4)
    njt = t // P  # key tiles per batch (16)
    nqb = t // 512  # query blocks per batch (4)

    nc = bacc.Bacc(
        "TRN2", target_bir_lowering=False, debug=False, num_devices=n_cores
    )

    # Kernel I/O (per-core tensors; host supplies per-core contents)
    hT_d = nc.dram_tensor("hT", [KCH, P, bt], BF16, kind="ExternalInput").ap()
    wq_d = nc.dram_tensor("wq", [KCH, P, P], BF16, kind="ExternalInput").ap()
    wk_d = nc.dram_tensor("wk", [KCH, P, P], BF16, kind="ExternalInput").ap()
    wv_d = nc.dram_tensor("wv", [KCH, P, P], BF16, kind="ExternalInput").ap()
    wo_d = nc.dram_tensor("wo", [KCH, P, D_MODEL], BF16, kind="ExternalInput").ap()
    hres_d = nc.dram_tensor("hres", [n_it, P, D_MODEL], F32, kind="ExternalInput").ap()
    g_d = nc.dram_tensor("lng", [D_MODEL], F32, kind="ExternalInput").ap()
    b_d = nc.dram_tensor("lnb", [D_MODEL], F32, kind="ExternalInput").ap()
    out_d = nc.dram_tensor("out", [n_it, P, D_MODEL], F32, kind="ExternalOutput").ap()
    if debug:
        avn_dbg = nc.dram_tensor("avn_dbg", [B * nh_loc, D_HEAD, t], BF16,
                                 kind="ExternalOutput").ap()
        sum_dbg = nc.dram_tensor("sum_dbg", [B * nh_loc, 1, t], F32,
                                 kind="ExternalOutput").ap()

    with tile.TileContext(nc) as tc:
        with (
            tc.tile_pool(name="consts", bufs=1) as consts,
            tc.tile_pool(name="ps", bufs=1, space="PSUM") as psp,
            tc.tile_pool(name="dram", bufs=1, space="DRAM") as dram,
        ):
            # ---- one-time constants ----
            wq_sb = consts.tile([P, KCH, P], BF16)
            wk_sb = consts.tile([P, KCH, P], BF16)
            wv_sb = consts.tile([P, KCH, P], BF16)
            wo_sb = consts.tile([P, KCH, D_MODEL], BF16)
            nc.sync.dma_start(out=wq_sb, in_=wq_d.transpose([1, 0, 2]))
            nc.sync.dma_start(out=wk_sb, in_=wk_d.transpose([1, 0, 2]))
            nc.sync.dma_start(out=wv_sb, in_=wv_d.transpose([1, 0, 2]))
            nc.sync.dma_start(out=wo_sb, in_=wo_d.transpose([1, 0, 2]))
            hres_sb = consts.tile([P, n_it, D_MODEL], F32)
            nc.sync.dma_start(out=hres_sb, in_=hres_d.transpose([1, 0, 2]))
            if apply_gb:
                g_sb = consts.tile([P, D_MODEL], F32)
                b_sb = consts.tile([P, D_MODEL], F32)
                nc.sync.dma_start(
                    out=g_sb,
                    in_=bass.AP(tensor=g_d.tensor, offset=g_d.offset, ap=[[0, P], *g_d.ap]),
                )
                nc.sync.dma_start(
                    out=b_sb,
                    in_=bass.AP(tensor=b_d.tensor, offset=b_d.offset, ap=[[0, P], *b_d.ap]),
                )

            eps_sb = consts.tile([P, 1], F32)
            nc.vector.memset(eps_sb, LN_EPS)
            expb_sb = consts.tile([P, 1], F32)
            nc.vector.memset(expb_sb, EXP_BIAS)

            # sliding causal mask: W[j, c] = 1.0 iff c >= 2048 + j, so the
            # window W[:, 2048 - jt*128 :][:, :2048] keeps (jt*128 + j) <= q
            W_sb = consts.tile([P, 2 * t], BF16)
            nc.gpsimd.memset(W_sb[:, 0 : t - P], 0.0)
            make_upper_triangular(nc, W_sb[:, t - P : t], val=1.0, diag=True)
            nc.gpsimd.memset(W_sb[:, t : 2 * t], 1.0)
            # NOTE: W[:, t-P : t] has 1 where j <= c-(t-P), i.e. the diagonal
            # block; window start offset for key tile jt is t - (jt+1)*128.

            # ---- persistent work tiles (written each repeat) ----
            h_sb = consts.tile([P, KCH, bt], BF16)       # h^T, dmodel-major
            wqstage = consts.tile([P, KCH, D_HEAD], BF16)  # current head's cols
            wkstage = consts.tile([P, KCH, D_HEAD], BF16)
            wvstage = consts.tile([P, KCH, D_HEAD], BF16)
            qstage = consts.tile([D_HEAD, t], BF16)      # current (b,h) queries
            hstage = consts.tile([P, KCH, P], BF16)
            astage = consts.tile([P, KCH, P], BF16)
            kstage = consts.tile([D_HEAD, P], BF16)
            vstage = consts.tile([P, D_HEAD + 1], BF16)
            nc.vector.memset(vstage[:, D_HEAD : D_HEAD + 1], 1.0)
            expt = consts.tile([P, t], BF16)
            srow = consts.tile([1, t], F32)
            rt = consts.tile([1, t], F32)
            rb = consts.tile([D_HEAD, t], F32)
            avn = consts.tile([D_HEAD, t], BF16)
            x_sb = consts.tile([P, D_MODEL], F32)
            xn_sb = consts.tile([P, D_MODEL], F32)
            stats = consts.tile([P, 2, 6], F32)
            mv = consts.tile([P, 2], F32)
            std = consts.tile([P, 1], F32)
            rstd = consts.tile([P, 1], F32)

            psA = psp.tile([P, 4 * 512], F32, tag="A")  # 4 banks
            psB = psp.tile([P, 4 * 512], F32, tag="B")  # 4 banks

            for _rep in range(repeat):
                # ---- load h^T ----
                nc.sync.dma_start(out=h_sb, in_=hT_d.transpose([1, 0, 2]))

                # ---- A2A buffers ----
                av_in = dram.tile([n_cores, P, B * cs], BF16,
                                  name=f"avin{_rep}")
                av_out = dram.tile([n_cores, P, B * cs], BF16,
                                   name=f"avout{_rep}")

                # ---- attention nest: batch (bi) -> head (hi) -> key tile (jt)
                # Q/K/V projections are folded in: Q for the whole batch is
                # computed per (bi, hi) before the key loop; K/V for one
                # 128-token tile are computed inside the key loop from a staged
                # h^T block (stationary operands must have static offsets, so
                # everything varying is staged through fixed SBUF tiles).
                with tc.For_i(0, B) as bi:
                    with tc.For_i(0, nh_loc) as hi:
                        for wst, wsb in ((wqstage, wq_sb), (wkstage, wk_sb),
                                         (wvstage, wv_sb)):
                            nc.sync.dma_start(
                                out=wst, in_=wsb[:, :, ds(hi * D_HEAD, D_HEAD)]
                            )
                        # Q for all nqb query blocks of this batch
                        qps = psA[0:D_HEAD, :]
                        for ib in range(nqb):
                            for k in range(KCH):
                                nc.tensor.matmul(
                                    qps[:, ib * 512 : (ib + 1) * 512],
                                    lhsT=wqstage[:, k, :],
                                    rhs=h_sb[:, k, ds(bi * t + ib * 512, 512)],
                                    start=(k == 0),
                                    stop=(k == KCH - 1),
                                )
                        nc.vector.tensor_copy(qstage, qps)
                        avps = psB[0 : D_HEAD + 1, :]
                        nc.vector.memset(avps, 0.0)
                        with tc.For_i(0, njt) as jt:
                            nc.sync.dma_start(
                                out=hstage, in_=h_sb[:, :, ds(jt * P + bi * t, P)]
                            )
                            kps = psA[0:D_HEAD, 0:P]
                            vps = psA[:, P : P + D_HEAD]
                            for k in range(KCH):
                                nc.tensor.matmul(
                                    kps,
                                    lhsT=wkstage[:, k, :],
                                    rhs=hstage[:, k, :],
                                    start=(k == 0),
                                    stop=(k == KCH - 1),
                                )
                            for k in range(KCH):
                                nc.tensor.matmul(
                                    vps,
                                    lhsT=hstage[:, k, :],
                                    rhs=wvstage[:, k, :],
                                    start=(k == 0),
                                    stop=(k == KCH - 1),
                                )
                            nc.vector.tensor_copy(kstage, kps)
                            nc.vector.tensor_copy(vstage[:, 0:D_HEAD], vps)
                            for ib in range(nqb):
                                nc.tensor.matmul(
                                    psA[:, ib * 512 : (ib + 1) * 512],
                                    lhsT=kstage,
                                    rhs=qstage[:, ib * 512 : (ib + 1) * 512],
                                    start=True,
                                    stop=True,
                                )
                            nc.scalar.activation(
                                expt, psA, mybir.ActivationFunctionType.Exp,
                                bias=expb_sb,
                            )
                            nc.vector.tensor_mul(
                                expt, expt, W_sb[:, ds(jt * (-P) + t - P, t)]
                            )
                            for ib in range(nqb):
                                nc.tensor.matmul(
                                    avps[:, ib * 512 : (ib + 1) * 512],
                                    lhsT=vstage,
                                    rhs=expt[:, ib * 512 : (ib + 1) * 512],
                                    start=False,
                                    stop=True,
                                    skip_group_check=True,
                                )
                        # normalize by sumexp (psum row 64) and ship to the
                        # A2A buffer: av_in[chunk, h*64+d, b*cs + t']
                        nc.vector.tensor_copy(srow, avps[D_HEAD : D_HEAD + 1, :])
                        nc.vector.reciprocal_approx_fast(out=rt, in_=srow)
                        nc.gpsimd.partition_broadcast(rb, rt)
                        nc.vector.tensor_mul(avn, avps[0:D_HEAD, :], rb)
                        nc.sync.dma_start(
                            out=av_in.transpose([1, 0, 2])[
                                ds(hi * D_HEAD, D_HEAD), :, :
                            ][:, :, ds(bi * cs, cs)],
                            in_=bass.AP(
                                tensor=avn.tensor,
                                offset=avn.offset,
                                ap=[[avn.ap[0][0], D_HEAD], [cs, n_cores], [1, cs]],
                            ),
                        )
                        if debug and _rep == 0:
                            nc.sync.dma_start(
                                out=avn_dbg[ds(bi * nh_loc + hi, 1)].squeeze(0),
                                in_=avn,
                            )
                            nc.sync.dma_start(
                                out=sum_dbg[ds(bi * nh_loc + hi, 1)].squeeze(0),
                                in_=srow,
                            )

                # ---- AllToAll ----
                if no_collective:
                    nc.sync.dma_start(out=av_out, in_=av_in)
                else:
                    nc.gpsimd.collective_compute(
                        "AllToAll",
                        mybir.AluOpType.bypass,
                        replica_groups=[list(range(n_cores))],
                        ins=[av_in.opt()],
                        outs=[av_out.opt()],
                    )

                # ---- output projection + residual + LayerNorm (unrolled: 4
                # static iterations cost less than For_i machinery) ----
                for i in range(n_it):
                    nc.sync.dma_start(
                        out=astage,
                        in_=av_out.transpose([1, 0, 2])[:, :, i * P : (i + 1) * P],
                    )
                    wops = psA[:, 0:1024]
                    for half in range(2):
                        for k in range(KCH):
                            nc.tensor.matmul(
                                wops[:, half * 512 : (half + 1) * 512],
                                lhsT=astage[:, k, :],
                                rhs=wo_sb[:, k, half * 512 : (half + 1) * 512],
                                start=(k == 0),
                                stop=(k == KCH - 1),
                            )
                    nc.vector.tensor_add(x_sb, wops, hres_sb[:, i, :])
                    for s in range(2):
                        nc.vector.bn_stats(stats[:, s, :], x_sb[:, s * 512 : (s + 1) * 512])
                    nc.vector.bn_aggr(mv, stats)
                    nc.scalar.activation(
                        std, mv[:, 1:2], mybir.ActivationFunctionType.Sqrt,
                        bias=eps_sb,
                    )
                    nc.vector.reciprocal(rstd, std)
                    nc.vector.tensor_scalar(
                        out=xn_sb,
                        in0=x_sb,
                        scalar1=mv[:, 0:1],
                        scalar2=rstd,
                        op0=mybir.AluOpType.subtract,
                        op1=mybir.AluOpType.mult,
                    )
                    if apply_gb:
                        nc.vector.tensor_mul(xn_sb, xn_sb, g_sb)
                        nc.vector.tensor_add(xn_sb, xn_sb, b_sb)
                    nc.sync.dma_start(out=out_d[i], in_=xn_sb)

    nc.compile()
    return nc


def make_in_maps(h, Wq, Wkv, Wo, ln_g, ln_b, t=T_FULL, n_cores=N_CORES):
    """Builds the per-core input maps (host-side sharding/layout prep)."""
    bfd = ml_dtypes.bfloat16
    nh_loc = N_HEAD // n_cores
    cs = t // n_cores
    n_it = B * cs // P

    # hT: [KCH, P, B*t] = h as [dmodel, batch-major tokens], bf16 (shared)
    hT = np.ascontiguousarray(
        h.transpose(2, 1, 0).reshape(KCH, P, B * t)
    ).astype(bfd)
    wo = np.ascontiguousarray(Wo).reshape(KCH, P, D_MODEL).astype(bfd)
    g = np.ascontiguousarray(ln_g, dtype=np.float32)
    bvec = np.ascontiguousarray(ln_b, dtype=np.float32)

    in_maps = []
    for c in range(n_cores):
        heads = [c * nh_loc + i for i in range(nh_loc)]
        wq_cols = np.concatenate(
            [Wq[:, hd * D_HEAD : (hd + 1) * D_HEAD] * SCALE for hd in heads], axis=1
        )
        wk_cols = np.concatenate(
            [Wkv[:, hd * 2 * D_HEAD : hd * 2 * D_HEAD + D_HEAD] for hd in heads],
            axis=1,
        )
        wv_cols = np.concatenate(
            [Wkv[:, hd * 2 * D_HEAD + D_HEAD : (hd + 1) * 2 * D_HEAD] for hd in heads],
            axis=1,
        )
        # residual rows for my token chunks, batch-major: it = b*2 + i2
        hres = np.concatenate(
            [
                h[b * 0 + c * cs : c * cs + cs, b, :].reshape(cs // P, P, D_MODEL)
                for b in range(B)
            ]
        ).reshape(n_it, P, D_MODEL)
        in_maps.append(
            {
                "hT": hT,
                "wq": np.ascontiguousarray(wq_cols.reshape(KCH, P, P)).astype(bfd),
                "wk": np.ascontiguousarray(wk_cols.reshape(KCH, P, P)).astype(bfd),
                "wv": np.ascontiguousarray(wv_cols.reshape(KCH, P, P)).astype(bfd),
                "wo": wo,
                "hres": np.ascontiguousarray(hres, dtype=np.float32),
                "lng": g,
                "lnb": bvec,
            }
        )
    return in_maps


def assemble_output(results, t=T_FULL, n_cores=N_CORES):
    cs = t // n_cores
    full = np.empty((t, B, D_MODEL), dtype=np.float32)
    for c in range(n_cores):
        o = results[c]["out"].reshape(B, cs, D_MODEL)
        for b in range(B):
            full[c * cs : (c + 1) * cs, b, :] = o[b]
    return full


def _numpy_fallback(h, attn_mask, Wq, Wkv, Wo, ln_g, ln_b):
    """Exact reference computation (only used if the mask is not causal)."""
    t, b, _ = h.shape
    hf = h.reshape(t * b, D_MODEL)
    q = (hf @ Wq).reshape(t, b, N_HEAD, D_HEAD)
    kv = (hf @ Wkv).reshape(t, b, N_HEAD, 2 * D_HEAD)
    k, v = kv[..., :D_HEAD], kv[..., D_HEAD:]
    s = np.einsum("ibnd,jbnd->ijbn", q, k) * SCALE
    s = np.where(attn_mask[:, :, :, None], -np.inf, s)
    s = s - s.max(axis=1, keepdims=True)
    p = np.exp(s)
    p = p / p.sum(axis=1, keepdims=True)
    av = np.einsum("ijbn,jbnd->ibnd", p, v).reshape(t, b, N_HEAD * D_HEAD)
    ao = av @ Wo
    x = h + ao
    mu = x.mean(axis=-1, keepdims=True)
    var = ((x - mu) ** 2).mean(axis=-1, keepdims=True)
    return ((x - mu) / np.sqrt(var + LN_EPS) * ln_g + ln_b).astype(np.float32)


_PROGRAM_CACHE = {}


def kernel(h, attn_mask, Wq, Wkv, Wo, ln_g, ln_b):
    global LAST_RESULT
    h = np.asarray(h, dtype=np.float32)
    attn_mask = np.asarray(attn_mask)
    Wq = np.asarray(Wq, dtype=np.float32)
    Wkv = np.asarray(Wkv, dtype=np.float32)
    Wo = np.asarray(Wo, dtype=np.float32)
    ln_g = np.asarray(ln_g, dtype=np.float32)
    ln_b = np.asarray(ln_b, dtype=np.float32)

    t = h.shape[0]
    causal = np.triu(np.ones((t, t), dtype=bool), k=1)
    if not np.array_equal(attn_mask, np.broadcast_to(causal[:, :, None], attn_mask.shape)):
        return _numpy_fallback(h, attn_mask, Wq, Wkv, Wo, ln_g, ln_b)

    apply_gb = not (np.all(ln_g == 1.0) and np.all(ln_b == 0.0))
    key = (t, apply_gb)
    if key not in _PROGRAM_CACHE:
        _PROGRAM_CACHE[key] = build_program(t=t, apply_gb=apply_gb)
    nc = _PROGRAM_CACHE[key]

    in_maps = make_in_maps(h, Wq, Wkv, Wo, ln_g, ln_b, t=t)
    res = run_bass_kernel_spmd(
        nc,
        in_maps,
        core_ids=list(range(N_CORES)),
        trace=bool(int(os.environ.get("KERNEL_TRACE", "0"))),
    )
    LAST_RESULT = res
    return assemble_output(res.results, t=t)


if __name__ == "__main__":
    build_program()
    print("program built ok")


# revision 27
# speedup vs baseline: 1.2109x; 1.2109x over previous
"""Trainium2 Bass kernel for nn_MultiHeadAttn (dense transformer block:
QKV proj -> causal MHA -> out proj -> residual -> LayerNorm).

Sharding: tensor-parallel over the 16 heads across 8 NeuronCores (2 heads
per core). Each core computes Q/K/V for its heads over all tokens, causal
attention with the softmax denominator carried as an appended ones-column
in V, then an AllToAll redistributes the per-head attention vectors so
each core holds all 16 heads for 1/8 of the token rows and applies the
output projection, residual add and LayerNorm for those rows.

The body is structured as a small number of For_i hardware loops with all
varying-operand matmul inputs staged through fixed SBUF addresses (the PE
stationary operand cannot take register offsets), which keeps the static
instruction count low:
  - one QK projection loop over 512-token blocks (weights stationary),
  - one V projection loop over 128-token tiles producing V token-major,
  - four (batch, head) attention loops over the 16 key tiles, computing
    scores for the full query range and masking via a sliding causal-mask
    window, accumulating AV in PSUM across iterations,
  - one output-projection + LayerNorm loop over 128-row output tiles.
"""

import os
import sys

import numpy as np

try:
    import concourse.bass as bass  # noqa: F401
except ImportError:  # pragma: no cover
    sys.path.insert(0, "/opt/trn_rl_repo")

import ml_dtypes

import concourse.bass as bass
from concourse.bass import ds, ts
import concourse.mybir as mybir
import concourse.tile as tile
from concourse import bacc
from concourse.bass_utils import run_bass_kernel_spmd
from concourse.masks import make_upper_triangular

# Problem constants
T_FULL = 2048
B = 2
D_MODEL = 1024
N_HEAD = 16
D_HEAD = 64
LN_EPS = 1e-5
N_CORES = 8
SCALE = 1.0 / (D_HEAD**0.5)
EXP_BIAS = -3.0  # scores are in [-3.3, 3.3] for this problem; keeps exp <= ~1.4

P = 128
KCH = D_MODEL // P  # 8 contraction chunks

F32 = mybir.dt.float32
BF16 = mybir.dt.bfloat16

# Stash of the most recent run's BassKernelResults (for test harnesses).
LAST_RESULT = None


def build_program(t=T_FULL, n_cores=N_CORES, repeat=1, no_collective=False, apply_gb=True,
                  debug=False):
    """Builds the SPMD Bass program (same program on every core).

    repeat > 1 re-emits the whole kernel body (everything except constant
    weight loads) that many times — used only for wall-clock timing.
    """
    nh_loc = N_HEAD // n_cores  # 2 heads per core
    assert nh_loc == 2
    bt = B * t  # flattened (batch, token) axis, batch-major
    cs = t // n_cores  # per-batch token chunk per core (A2A shard)
    n_it = B * cs // P  # 128-row output tiles per core (4)
    njt = t // P  # key tiles per batch (16)
    nqb = t // 512  # query blocks per batch (4)

    nc = bacc.Bacc(
        "TRN2", target_bir_lowering=False, debug=False, num_devices=n_cores
    )

    # Kernel I/O (per-core tensors; host supplies per-core contents)
    hT_d = nc.dram_tensor("hT", [KCH, P, bt], BF16, kind="ExternalInput").ap()
    wq_d = nc.dram_tensor("wq", [KCH, P, P], BF16, kind="ExternalInput").ap()
    wk_d = nc.dram_tensor("wk", [KCH, P, P], BF16, kind="ExternalInput").ap()
    wv_d = nc.dram_tensor("wv", [KCH, P, P], BF16, kind="ExternalInput").ap()
    wo_d = nc.dram_tensor("wo", [KCH, P, D_MODEL], BF16, kind="ExternalInput").ap()
    hres_d = nc.dram_tensor("hres", [n_it, P, D_MODEL], F32, kind="ExternalInput").ap()
    g_d = nc.dram_tensor("lng", [D_MODEL], F32, kind="ExternalInput").ap()
    b_d = nc.dram_tensor("lnb", [D_MODEL], F32, kind="ExternalInput").ap()
    out_d = nc.dram_tensor("out", [n_it, P, D_MODEL], F32, kind="ExternalOutput").ap()
    if debug:
        avn_dbg = nc.dram_tensor("avn_dbg", [B * nh_loc, D_HEAD, t], BF16,
                                 kind="ExternalOutput").ap()
        sum_dbg = nc.dram_tensor("sum_dbg", [B * nh_loc, 1, t], F32,
                                 kind="ExternalOutput").ap()

    with tile.TileContext(nc) as tc:
        with (
            tc.tile_pool(name="consts", bufs=1) as consts,
            tc.tile_pool(name="ps", bufs=1, space="PSUM") as psp,
            tc.tile_pool(name="dram", bufs=1, space="DRAM") as dram,
        ):
            # ---- one-time constants ----
            wq_sb = consts.tile([P, KCH, P], BF16)
            wk_sb = consts.tile([P, KCH, P], BF16)
            wv_sb = consts.tile([P, KCH, P], BF16)
            wo_sb = consts.tile([P, KCH, D_MODEL], BF16)
            nc.sync.dma_start(out=wq_sb, in_=wq_d.transpose([1, 0, 2]))
            nc.sync.dma_start(out=wk_sb, in_=wk_d.transpose([1, 0, 2]))
            nc.sync.dma_start(out=wv_sb, in_=wv_d.transpose([1, 0, 2]))
            nc.sync.dma_start(out=wo_sb, in_=wo_d.transpose([1, 0, 2]))
            hres_sb = consts.tile([P, n_it, D_MODEL], F32)
            nc.sync.dma_start(out=hres_sb, in_=hres_d.transpose([1, 0, 2]))
            if apply_gb:
                g_sb = consts.tile([P, D_MODEL], F32)
                b_sb = consts.tile([P, D_MODEL], F32)
                nc.sync.dma_start(
                    out=g_sb,
                    in_=bass.AP(tensor=g_d.tensor, offset=g_d.offset, ap=[[0, P], *g_d.ap]),
                )
                nc.sync.dma_start(
                    out=b_sb,
                    in_=bass.AP(tensor=b_d.tensor, offset=b_d.offset, ap=[[0, P], *b_d.ap]),
                )

            eps_sb = consts.tile([P, 1], F32)
            nc.vector.memset(eps_sb, LN_EPS)
            expb_sb = consts.tile([P, 1], F32)
            nc.vector.memset(expb_sb, EXP_BIAS)

            # sliding causal mask: W[j, c] = 1.0 iff c >= 2048 + j, so the
            # window W[:, 2048 - jt*128 :][:, :2048] keeps (jt*128 + j) <= q
            W_sb = consts.tile([P, 2 * t], BF16)
            nc.gpsimd.memset(W_sb[:, 0 : t - P], 0.0)
            make_upper_triangular(nc, W_sb[:, t - P : t], val=1.0, diag=True)
            nc.gpsimd.memset(W_sb[:, t : 2 * t], 1.0)
            # NOTE: W[:, t-P : t] has 1 where j <= c-(t-P), i.e. the diagonal
            # block; window start offset for key tile jt is t - (jt+1)*128.

            # ---- persistent work tiles (written each repeat) ----
            h_sb = consts.tile([P, KCH, bt], BF16)       # h^T, dmodel-major
            qT_sb = consts.tile([D_HEAD, nh_loc, bt], BF16)
            kT_sb = consts.tile([D_HEAD, nh_loc, bt], BF16)
            vext = consts.tile([P, B * njt, nh_loc, D_HEAD + 1], BF16)
            nc.vector.memset(vext[:, :, :, D_HEAD : D_HEAD + 1], 1.0)
            hstage = consts.tile([P, KCH, P], BF16)
            avg_sb = consts.tile([P, KCH, B * cs], BF16)
            kstage = consts.tile([D_HEAD, P], BF16)
            vstage = consts.tile([P, D_HEAD + 1], BF16)
            nc.vector.memset(vstage[:, D_HEAD : D_HEAD + 1], 1.0)
            expt = consts.tile([P, t], BF16)
            srow = consts.tile([1, t], F32)
            rt = srow
            rb = consts.tile([D_HEAD, t], F32)
            avn = consts.tile([D_HEAD, t], BF16)
            x_sb = consts.tile([P, D_MODEL], F32)
            xn_sb = consts.tile([P, D_MODEL], F32)
            stats = consts.tile([P, 2, 6], F32)
            mv = consts.tile([P, 2], F32)
            std = consts.tile([P, 1], F32)
            rstd = consts.tile([P, 1], F32)

            psA = psp.tile([P, 4 * 512], F32, tag="A")  # 4 banks
            psB = psp.tile([P, 4 * 512], F32, tag="B")  # 4 banks

            for _rep in range(repeat):
                # ---- load h^T ----
                nc.sync.dma_start(out=h_sb, in_=hT_d.transpose([1, 0, 2]))

                # ---- A2A buffers ----
                av_in = dram.tile([n_cores, P, B * cs], BF16,
                                  name=f"avin{_rep}")
                av_out = dram.tile([n_cores, P, B * cs], BF16,
                                   name=f"avout{_rep}")

                # ---- QK projections: loop over 512-token blocks ----
                with tc.For_i(0, bt // 512) as i:
                    for mt, (wsb, dst) in enumerate(((wq_sb, qT_sb), (wk_sb, kT_sb))):
                        pslice = psA[:, mt * 512 : (mt + 1) * 512]
                        for k in range(KCH):
                            nc.tensor.matmul(
                                pslice,
                                lhsT=wsb[:, k, :],
                                rhs=h_sb[:, k, ts(i, 512)],
                                start=(k == 0),
                                stop=(k == KCH - 1),
                            )
                        for hl in range(nh_loc):
                            nc.vector.tensor_copy(
                                dst[:, hl, ts(i, 512)],
                                pslice[hl * D_HEAD : (hl + 1) * D_HEAD, :],
                            )

                # ---- V projection token-major: loop over 128-token tiles ----
                with tc.For_i(0, bt // P) as i:
                    nc.sync.dma_start(out=hstage, in_=h_sb[:, :, ts(i, P)])
                    vps = psA[:, 2 * 512 : 2 * 512 + P]
                    for k in range(KCH):
                        nc.tensor.matmul(
                            vps,
                            lhsT=hstage[:, k, :],
                            rhs=wv_sb[:, k, :],
                            start=(k == 0),
                            stop=(k == KCH - 1),
                        )
                    nc.vector.tensor_copy(
                        vext[:, ds(i, 1), :, 0:D_HEAD].squeeze(1), vps
                    )

                # ---- attention: per (batch, head), loop over key tiles ----
                for b in range(B):
                    for h in range(nh_loc):
                        pbase = h * D_HEAD
                        avps = psB[0 : D_HEAD + 1, :]
                        nc.vector.memset(avps, 0.0)
                        with tc.For_i(0, njt) as jt:
                            nc.sync.dma_start(
                                out=kstage,
                                in_=kT_sb[:, h, ds(jt * P + b * t, P)],
                            )
                            nc.sync.dma_start(
                                out=vstage,
                                in_=vext[:, ds(jt + b * njt, 1), h, :].squeeze(1),
                            )
                            for ib in range(nqb):
                                nc.tensor.matmul(
                                    psA[:, ib * 512 : (ib + 1) * 512],
                                    lhsT=kstage,
                                    rhs=qT_sb[:, h,
                                              b * t + ib * 512 : b * t + (ib + 1) * 512],
                                    start=True,
                                    stop=True,
                                )
                            nc.scalar.activation(
                                expt, psA, mybir.ActivationFunctionType.Exp,
                                bias=expb_sb,
                            )
                            nc.vector.tensor_mul(
                                expt, expt, W_sb[:, ds(jt * (-P) + t - P, t)]
                            )
                            for ib in range(nqb):
                                nc.tensor.matmul(
                                    avps[:, ib * 512 : (ib + 1) * 512],
                                    lhsT=vstage,
                                    rhs=expt[:, ib * 512 : (ib + 1) * 512],
                                    start=False,
                                    stop=True,
                                    skip_group_check=True,
                                )
                        nc.vector.tensor_copy(srow, avps[D_HEAD : D_HEAD + 1, :])
                        nc.vector.reciprocal_approx_fast(out=rt, in_=srow)  # in-place
                        nc.gpsimd.partition_broadcast(rb, rt)
                        nc.vector.tensor_mul(avn, avps[0:D_HEAD, :], rb)
                        nc.sync.dma_start(
                            out=av_in.transpose([1, 0, 2])[
                                pbase : pbase + D_HEAD, :, :
                            ][:, :, b * cs : (b + 1) * cs],
                            in_=bass.AP(
                                tensor=avn.tensor,
                                offset=avn.offset,
                                ap=[[avn.ap[0][0], D_HEAD], [cs, n_cores], [1, cs]],
                            ),
                        )

                # ---- AllToAll ----
                if no_collective:
                    nc.sync.dma_start(out=av_out, in_=av_in)
                else:
                    nc.gpsimd.collective_compute(
                        "AllToAll",
                        mybir.AluOpType.bypass,
                        replica_groups=[list(range(n_cores))],
                        ins=[av_in.opt()],
                        outs=[av_out.opt()],
                    )

                # ---- output projection + residual + LayerNorm (unrolled: 4
                # static iterations cost less than For_i machinery; static
                # offsets also let the matmuls read avg_sb directly) ----
                nc.sync.dma_start(out=avg_sb, in_=av_out.transpose([1, 0, 2]))
                for i in range(n_it):
                    wops = psA[:, 0:1024]
                    for half in range(2):
                        for k in range(KCH):
                            nc.tensor.matmul(
                                wops[:, half * 512 : (half + 1) * 512],
                                lhsT=avg_sb[:, k, i * P : (i + 1) * P],
                                rhs=wo_sb[:, k, half * 512 : (half + 1) * 512],
                                start=(k == 0),
                                stop=(k == KCH - 1),
                            )
                    nc.vector.tensor_add(x_sb, wops, hres_sb[:, i, :])
                    for s in range(2):
                        nc.vector.bn_stats(stats[:, s, :], x_sb[:, s * 512 : (s + 1) * 512])
                    nc.vector.bn_aggr(mv, stats)
                    nc.scalar.activation(
                        std, mv[:, 1:2], mybir.ActivationFunctionType.Sqrt,
                        bias=eps_sb,
                    )
                    nc.vector.reciprocal(rstd, std)
                    nc.vector.tensor_scalar(
                        out=xn_sb,
                        in0=x_sb,
                        scalar1=mv[:, 0:1],
                        scalar2=rstd,
                        op0=mybir.AluOpType.subtract,
                        op1=mybir.AluOpType.mult,
                    )
                    if apply_gb:
                        nc.vector.tensor_mul(xn_sb, xn_sb, g_sb)
                        nc.vector.tensor_add(xn_sb, xn_sb, b_sb)
                    nc.sync.dma_start(out=out_d[i], in_=xn_sb)

    nc.compile()
    return nc


def make_in_maps(h, Wq, Wkv, Wo, ln_g, ln_b, t=T_FULL, n_cores=N_CORES):
    """Builds the per-core input maps (host-side sharding/layout prep)."""
    bfd = ml_dtypes.bfloat16
    nh_loc = N_HEAD // n_cores
    cs = t // n_cores
    n_it = B * cs // P

    # hT: [KCH, P, B*t] = h as [dmodel, batch-major tokens], bf16 (shared)
    hT = np.ascontiguousarray(
        h.transpose(2, 1, 0).reshape(KCH, P, B * t)
    ).astype(bfd)
    wo = np.ascontiguousarray(Wo).reshape(KCH, P, D_MODEL).astype(bfd)
    g = np.ascontiguousarray(ln_g, dtype=np.float32)
    bvec = np.ascontiguousarray(ln_b, dtype=np.float32)

    in_maps = []
    for c in range(n_cores):
        heads = [c * nh_loc + i for i in range(nh_loc)]
        wq_cols = np.concatenate(
            [Wq[:, hd * D_HEAD : (hd + 1) * D_HEAD] * SCALE for hd in heads], axis=1
        )
        wk_cols = np.concatenate(
            [Wkv[:, hd * 2 * D_HEAD : hd * 2 * D_HEAD + D_HEAD] for hd in heads],
            axis=1,
        )
        wv_cols = np.concatenate(
            [Wkv[:, hd * 2 * D_HEAD + D_HEAD : (hd + 1) * 2 * D_HEAD] for hd in heads],
            axis=1,
        )
        # residual rows for my token chunks, batch-major: it = b*2 + i2
        hres = np.concatenate(
            [
                h[b * 0 + c * cs : c * cs + cs, b, :].reshape(cs // P, P, D_MODEL)
                for b in range(B)
            ]
        ).reshape(n_it, P, D_MODEL)
        in_maps.append(
            {
                "hT": hT,
                "wq": np.ascontiguousarray(wq_cols.reshape(KCH, P, P)).astype(bfd),
                "wk": np.ascontiguousarray(wk_cols.reshape(KCH, P, P)).astype(bfd),
                "wv": np.ascontiguousarray(wv_cols.reshape(KCH, P, P)).astype(bfd),
                "wo": wo,
                "hres": np.ascontiguousarray(hres, dtype=np.float32),
                "lng": g,
                "lnb": bvec,
            }
        )
    return in_maps


def assemble_output(results, t=T_FULL, n_cores=N_CORES):
    cs = t // n_cores
    full = np.empty((t, B, D_MODEL), dtype=np.float32)
    for c in range(n_cores):
        o = results[c]["out"].reshape(B, cs, D_MODEL)
        for b in range(B):
            full[c * cs : (c + 1) * cs, b, :] = o[b]
    return full


def _numpy_fallback(h, attn_mask, Wq, Wkv, Wo, ln_g, ln_b):
    """Exact reference computation (only used if the mask is not causal)."""
    t, b, _ = h.shape
    hf = h.reshape(t * b, D_MODEL)
    q = (hf @ Wq).reshape(t, b, N_HEAD, D_HEAD)
    kv = (hf @ Wkv).reshape(t, b, N_HEAD, 2 * D_HEAD)
    k, v = kv[..., :D_HEAD], kv[..., D_HEAD:]
    s = np.einsum("ibnd,jbnd->ijbn", q, k) * SCALE
    s = np.where(attn_mask[:, :, :, None], -np.inf, s)
    s = s - s.max(axis=1, keepdims=True)
    p = np.exp(s)
    p = p / p.sum(axis=1, keepdims=True)
    av = np.einsum("ijbn,jbnd->ibnd", p, v).reshape(t, b, N_HEAD * D_HEAD)
    ao = av @ Wo
    x = h + ao
    mu = x.mean(axis=-1, keepdims=True)
    var = ((x - mu) ** 2).mean(axis=-1, keepdims=True)
    return ((x - mu) / np.sqrt(var + LN_EPS) * ln_g + ln_b).astype(np.float32)


_PROGRAM_CACHE = {}


def kernel(h, attn_mask, Wq, Wkv, Wo, ln_g, ln_b):
    global LAST_RESULT
    h = np.asarray(h, dtype=np.float32)
    attn_mask = np.asarray(attn_mask)
    Wq = np.asarray(Wq, dtype=np.float32)
    Wkv = np.asarray(Wkv, dtype=np.float32)
    Wo = np.asarray(Wo, dtype=np.float32)
    ln_g = np.asarray(ln_g, dtype=np.float32)
    ln_b = np.asarray(ln_b, dtype=np.float32)

    t = h.shape[0]
    causal = np.triu(np.ones((t, t), dtype=bool), k=1)
    if not np.array_equal(attn_mask, np.broadcast_to(causal[:, :, None], attn_mask.shape)):
        return _numpy_fallback(h, attn_mask, Wq, Wkv, Wo, ln_g, ln_b)

    apply_gb = not (np.all(ln_g == 1.0) and np.all(ln_b == 0.0))
    key = (t, apply_gb)
    if key not in _PROGRAM_CACHE:
        _PROGRAM_CACHE[key] = build_program(t=t, apply_gb=apply_gb)
    nc = _PROGRAM_CACHE[key]

    in_maps = make_in_maps(h, Wq, Wkv, Wo, ln_g, ln_b, t=t)
    res = run_bass_kernel_spmd(
        nc,
        in_maps,
        core_ids=list(range(N_CORES)),
        trace=bool(int(os.environ.get("KERNEL_TRACE", "0"))),
    )
    LAST_RESULT = res
    return assemble_output(res.results, t=t)


if __name__ == "__main__":
    build_program()
    print("program built ok")


# revision 29
# speedup vs baseline: 1.3764x; 1.1367x over previous
"""Trainium2 Bass kernel for nn_MultiHeadAttn (dense transformer block:
QKV proj -> causal MHA -> out proj -> residual -> LayerNorm).

Sharding: tensor-parallel over the 16 heads across 8 NeuronCores (2 heads
per core). Each core computes Q/K/V for its heads over all tokens, causal
attention with the softmax denominator carried as an appended ones-column
in V, then an AllToAll redistributes the per-head attention vectors so
each core holds all 16 heads for 1/8 of the token rows and applies the
output projection, residual add and LayerNorm for those rows.

The body is structured as a small number of For_i hardware loops with all
varying-operand matmul inputs staged through fixed SBUF addresses (the PE
stationary operand cannot take register offsets), which keeps the static
instruction count low:
  - one QK projection loop over 512-token blocks (weights stationary),
  - one V projection loop over 128-token tiles producing V token-major
    with an appended ones-column (sumexp via the AV matmul),
  - four (batch, head) attention loops over the 16 key tiles: scores are
    computed for the full query range of the batch and masked after exp
    via a sliding causal-mask window read at a loop-dependent offset;
    AV accumulates in pre-zeroed PSUM across iterations (start=False),
  - an unrolled output-projection + LayerNorm tail over the 4 output
    row-tiles, reading the gathered AllToAll result directly as the
    stationary operand (static offsets).
"""

import os
import sys

import numpy as np

try:
    import concourse.bass as bass  # noqa: F401
except ImportError:  # pragma: no cover
    sys.path.insert(0, "/opt/trn_rl_repo")

import ml_dtypes

import concourse.bass as bass
from concourse.bass import ds, ts
import concourse.mybir as mybir
import concourse.tile as tile
from concourse import bacc
from concourse.bass_utils import run_bass_kernel_spmd
from concourse.masks import make_upper_triangular

# Problem constants
T_FULL = 2048
B = 2
D_MODEL = 1024
N_HEAD = 16
D_HEAD = 64
LN_EPS = 1e-5
N_CORES = 8
SCALE = 1.0 / (D_HEAD**0.5)
EXP_BIAS = -3.0  # scores are in [-3.3, 3.3] for this problem; keeps exp <= ~1.4

P = 128
KCH = D_MODEL // P  # 8 contraction chunks

F32 = mybir.dt.float32
BF16 = mybir.dt.bfloat16

# Stash of the most recent run's BassKernelResults (for test harnesses).
LAST_RESULT = None


def build_program(t=T_FULL, n_cores=N_CORES, repeat=1, no_collective=False, apply_gb=True,
                  debug=False):
    """Builds the SPMD Bass program (same program on every core).

    repeat > 1 re-emits the whole kernel body (everything except constant
    weight loads) that many times — used only for wall-clock timing.
    """
    nh_loc = N_HEAD // n_cores  # 2 heads per core
    assert nh_loc == 2
    bt = B * t  # flattened (batch, token) axis, batch-major
    cs = t // n_cores  # per-batch token chunk per core (A2A shard)
    n_it = B * cs // P  # 128-row output tiles per core (4)
    njt = t // P  # key tiles per batch (16)
    nqb = t // 512  # query blocks per batch (4)

    nc = bacc.Bacc(
        "TRN2", target_bir_lowering=False, debug=False, num_devices=n_cores
    )

    # Kernel I/O (per-core tensors; host supplies per-core contents)
    hT_d = nc.dram_tensor("hT", [KCH, P, bt], BF16, kind="ExternalInput").ap()
    wq_d = nc.dram_tensor("wq", [KCH, P, P], BF16, kind="ExternalInput").ap()
    wk_d = nc.dram_tensor("wk", [KCH, P, P], BF16, kind="ExternalInput").ap()
    wv_d = nc.dram_tensor("wv", [KCH, P, P], BF16, kind="ExternalInput").ap()
    wo_d = nc.dram_tensor("wo", [KCH, P, D_MODEL], BF16, kind="ExternalInput").ap()
    hres_d = nc.dram_tensor("hres", [n_it, P, D_MODEL], F32, kind="ExternalInput").ap()
    g_d = nc.dram_tensor("lng", [D_MODEL], F32, kind="ExternalInput").ap()
    b_d = nc.dram_tensor("lnb", [D_MODEL], F32, kind="ExternalInput").ap()
    out_d = nc.dram_tensor("out", [n_it, P, D_MODEL], F32, kind="ExternalOutput").ap()
    if debug:
        avn_dbg = nc.dram_tensor("avn_dbg", [B * nh_loc, D_HEAD, t], BF16,
                                 kind="ExternalOutput").ap()
        sum_dbg = nc.dram_tensor("sum_dbg", [B * nh_loc, 1, t], F32,
                                 kind="ExternalOutput").ap()

    with tile.TileContext(nc) as tc:
        with (
            tc.tile_pool(name="consts", bufs=1) as consts,
            tc.tile_pool(name="ps", bufs=1, space="PSUM") as psp,
            tc.tile_pool(name="dram", bufs=1, space="DRAM") as dram,
        ):
            # ---- one-time constants ----
            wq_sb = consts.tile([P, KCH, P], BF16)
            wk_sb = consts.tile([P, KCH, P], BF16)
            wv_sb = consts.tile([P, KCH, P], BF16)
            wo_sb = consts.tile([P, KCH, D_MODEL], BF16)
            nc.sync.dma_start(out=wq_sb, in_=wq_d.transpose([1, 0, 2]))
            nc.sync.dma_start(out=wk_sb, in_=wk_d.transpose([1, 0, 2]))
            nc.sync.dma_start(out=wv_sb, in_=wv_d.transpose([1, 0, 2]))
            nc.sync.dma_start(out=wo_sb, in_=wo_d.transpose([1, 0, 2]))
            hres_sb = consts.tile([P, n_it, D_MODEL], F32)
            nc.sync.dma_start(out=hres_sb, in_=hres_d.transpose([1, 0, 2]))
            if apply_gb:
                g_sb = consts.tile([P, D_MODEL], F32)
                b_sb = consts.tile([P, D_MODEL], F32)
                nc.sync.dma_start(
                    out=g_sb,
                    in_=bass.AP(tensor=g_d.tensor, offset=g_d.offset, ap=[[0, P], *g_d.ap]),
                )
                nc.sync.dma_start(
                    out=b_sb,
                    in_=bass.AP(tensor=b_d.tensor, offset=b_d.offset, ap=[[0, P], *b_d.ap]),
                )

            eps_sb = consts.tile([P, 1], F32)
            nc.vector.memset(eps_sb, LN_EPS)
            expb_sb = consts.tile([P, 1], F32)
            nc.vector.memset(expb_sb, EXP_BIAS)

            # sliding causal mask: W[j, c] = 1.0 iff c >= 2048 + j, so the
            # window W[:, 2048 - jt*128 :][:, :2048] keeps (jt*128 + j) <= q
            W_sb = consts.tile([P, 2 * t], BF16)
            nc.gpsimd.memset(W_sb[:, 0 : t - P], 0.0)
            make_upper_triangular(nc, W_sb[:, t - P : t], val=1.0, diag=True)
            nc.gpsimd.memset(W_sb[:, t : 2 * t], 1.0)
            # NOTE: W[:, t-P : t] has 1 where j <= c-(t-P), i.e. the diagonal
            # block; window start offset for key tile jt is t - (jt+1)*128.

            # ---- persistent work tiles (written each repeat) ----
            h_sb = consts.tile([P, KCH, bt], BF16)       # h^T, dmodel-major
            qT_sb = consts.tile([D_HEAD, nh_loc, bt], BF16)
            kT_sb = consts.tile([D_HEAD, nh_loc, bt], BF16)
            vext = consts.tile([P, B * njt, nh_loc, D_HEAD + 1], BF16)
            nc.vector.memset(vext[:, :, :, D_HEAD : D_HEAD + 1], 1.0)
            hstage = consts.tile([P, KCH, P], BF16)
            avg_sb = consts.tile([P, KCH, B * cs], BF16)
            kstage = consts.tile([D_HEAD, P], BF16)
            vstage = consts.tile([P, D_HEAD + 1], BF16)
            nc.vector.memset(vstage[:, D_HEAD : D_HEAD + 1], 1.0)
            expt = consts.tile([P, t], BF16)
            srow = consts.tile([1, t], F32)
            rt = srow
            rb = consts.tile([D_HEAD, t], F32)
            avn = consts.tile([D_HEAD, t], BF16)
            x_sb = consts.tile([P, D_MODEL], F32)
            xn_sb = consts.tile([P, D_MODEL], F32)
            stats = consts.tile([P, 2, 6], F32)
            mv = consts.tile([P, 2], F32)
            std = consts.tile([P, 1], F32)
            rstd = consts.tile([P, 1], F32)

            psA = psp.tile([P, 4 * 512], F32, tag="A")  # 4 banks
            psB = psp.tile([P, 4 * 512], F32, tag="B")  # 4 banks

            for _rep in range(repeat):
                # ---- load h^T ----
                nc.sync.dma_start(out=h_sb, in_=hT_d.transpose([1, 0, 2]))

                # ---- A2A buffers ----
                av_in = dram.tile([n_cores, P, B * cs], BF16,
                                  name=f"avin{_rep}")
                av_out = dram.tile([n_cores, P, B * cs], BF16,
                                   name=f"avout{_rep}")

                # ---- QK projections: loop over 512-token blocks ----
                with tc.For_i(0, bt // 512) as i:
                    for mt, (wsb, dst) in enumerate(((wq_sb, qT_sb), (wk_sb, kT_sb))):
                        pslice = psA[:, mt * 512 : (mt + 1) * 512]
                        for k in range(KCH):
                            nc.tensor.matmul(
                                pslice,
                                lhsT=wsb[:, k, :],
                                rhs=h_sb[:, k, ts(i, 512)],
                                start=(k == 0),
                                stop=(k == KCH - 1),
                            )
                        for hl in range(nh_loc):
                            nc.vector.tensor_copy(
                                dst[:, hl, ts(i, 512)],
                                pslice[hl * D_HEAD : (hl + 1) * D_HEAD, :],
                            )

                # ---- V projection token-major: loop over 128-token tiles ----
                with tc.For_i(0, bt // P) as i:
                    nc.sync.dma_start(out=hstage, in_=h_sb[:, :, ts(i, P)])
                    vps = psA[:, 2 * 512 : 2 * 512 + P]
                    for k in range(KCH):
                        nc.tensor.matmul(
                            vps,
                            lhsT=hstage[:, k, :],
                            rhs=wv_sb[:, k, :],
                            start=(k == 0),
                            stop=(k == KCH - 1),
                        )
                    nc.vector.tensor_copy(
                        vext[:, ds(i, 1), :, 0:D_HEAD].squeeze(1), vps
                    )

                # ---- attention: nested loops batch -> head -> key tile ----
                with tc.For_i(0, B) as bi:
                    with tc.For_i(0, nh_loc) as hi:
                        avps = psB[0 : D_HEAD + 1, :]
                        nc.vector.memset(avps, 0.0)
                        with tc.For_i(0, njt) as jt:
                            nc.sync.dma_start(
                                out=kstage,
                                in_=kT_sb[:, ds(hi, 1), :].squeeze(1)[
                                    :, ds(jt * P + bi * t, P)],
                            )
                            nc.sync.dma_start(
                                out=vstage,
                                in_=vext[:, ds(jt + bi * njt, 1), :, :].squeeze(1)[
                                    :, ds(hi, 1), :].squeeze(1),
                            )
                            for ib in range(nqb):
                                nc.tensor.matmul(
                                    psA[:, ib * 512 : (ib + 1) * 512],
                                    lhsT=kstage,
                                    rhs=qT_sb[:, ds(hi, 1), :].squeeze(1)[
                                        :, ds(bi * t + ib * 512, 512)],
                                    start=True,
                                    stop=True,
                                )
                            nc.scalar.activation(
                                expt, psA, mybir.ActivationFunctionType.Exp,
                                bias=expb_sb,
                            )
                            nc.vector.tensor_mul(
                                expt, expt, W_sb[:, ds(jt * (-P) + t - P, t)]
                            )
                            for ib in range(nqb):
                                nc.tensor.matmul(
                                    avps[:, ib * 512 : (ib + 1) * 512],
                                    lhsT=vstage,
                                    rhs=expt[:, ib * 512 : (ib + 1) * 512],
                                    start=False,
                                    stop=True,
                                    skip_group_check=True,
                                )
                        nc.vector.tensor_copy(srow, avps[D_HEAD : D_HEAD + 1, :])
                        nc.vector.reciprocal_approx_fast(out=rt, in_=srow)  # in-place
                        nc.gpsimd.partition_broadcast(rb, rt)
                        nc.vector.tensor_mul(avn, avps[0:D_HEAD, :], rb)
                        nc.sync.dma_start(
                            out=av_in.transpose([1, 0, 2])[
                                ds(hi * D_HEAD, D_HEAD), :, :
                            ][:, :, ds(bi * cs, cs)],
                            in_=bass.AP(
                                tensor=avn.tensor,
                                offset=avn.offset,
                                ap=[[avn.ap[0][0], D_HEAD], [cs, n_cores], [1, cs]],
                            ),
                        )

                # ---- AllToAll ----
                if no_collective:
                    nc.sync.dma_start(out=av_out, in_=av_in)
                else:
                    nc.gpsimd.collective_compute(
                        "AllToAll",
                        mybir.AluOpType.bypass,
                        replica_groups=[list(range(n_cores))],
                        ins=[av_in.opt()],
                        outs=[av_out.opt()],
                    )

                # ---- output projection + residual + LayerNorm (unrolled: 4
                # static iterations cost less than For_i machinery; static
                # offsets also let the matmuls read avg_sb directly) ----
                nc.sync.dma_start(out=avg_sb, in_=av_out.transpose([1, 0, 2]))
                for i in range(n_it):
                    wops = psA[:, 0:1024]
                    for half in range(2):
                        for k in range(KCH):
                            nc.tensor.matmul(
                                wops[:, half * 512 : (half + 1) * 512],
                                lhsT=avg_sb[:, k, i * P : (i + 1) * P],
                                rhs=wo_sb[:, k, half * 512 : (half + 1) * 512],
                                start=(k == 0),
                                stop=(k == KCH - 1),
                            )
                    nc.vector.tensor_add(x_sb, wops, hres_sb[:, i, :])
                    for s in range(2):
                        nc.vector.bn_stats(stats[:, s, :], x_sb[:, s * 512 : (s + 1) * 512])
                    nc.vector.bn_aggr(mv, stats)
                    nc.scalar.activation(
                        std, mv[:, 1:2], mybir.ActivationFunctionType.Sqrt,
                        bias=eps_sb,
                    )
                    nc.vector.reciprocal(rstd, std)
                    nc.vector.tensor_scalar(
                        out=xn_sb,
                        in0=x_sb,
                        scalar1=mv[:, 0:1],
                        scalar2=rstd,
                        op0=mybir.AluOpType.subtract,
                        op1=mybir.AluOpType.mult,
                    )
                    if apply_gb:
                        nc.vector.tensor_mul(xn_sb, xn_sb, g_sb)
                        nc.vector.tensor_add(xn_sb, xn_sb, b_sb)
                    nc.sync.dma_start(out=out_d[i], in_=xn_sb)

    nc.compile()
    return nc


def make_in_maps(h, Wq, Wkv, Wo, ln_g, ln_b, t=T_FULL, n_cores=N_CORES):
    """Builds the per-core input maps (host-side sharding/layout prep)."""
    bfd = ml_dtypes.bfloat16
    nh_loc = N_HEAD // n_cores
    cs = t // n_cores
    n_it = B * cs // P

    # hT: [KCH, P, B*t] = h as [dmodel, batch-major tokens], bf16 (shared)
    hT = np.ascontiguousarray(
        h.transpose(2, 1, 0).reshape(KCH, P, B * t)
    ).astype(bfd)
    wo = np.ascontiguousarray(Wo).reshape(KCH, P, D_MODEL).astype(bfd)
    g = np.ascontiguousarray(ln_g, dtype=np.float32)
    bvec = np.ascontiguousarray(ln_b, dtype=np.float32)

    in_maps = []
    for c in range(n_cores):
        heads = [c * nh_loc + i for i in range(nh_loc)]
        wq_cols = np.concatenate(
            [Wq[:, hd * D_HEAD : (hd + 1) * D_HEAD] * SCALE for hd in heads], axis=1
        )
        wk_cols = np.concatenate(
            [Wkv[:, hd * 2 * D_HEAD : hd * 2 * D_HEAD + D_HEAD] for hd in heads],
            axis=1,
        )
        wv_cols = np.concatenate(
            [Wkv[:, hd * 2 * D_HEAD + D_HEAD : (hd + 1) * 2 * D_HEAD] for hd in heads],
            axis=1,
        )
        # residual rows for my token chunks, batch-major: it = b*2 + i2
        hres = np.concatenate(
            [
                h[b * 0 + c * cs : c * cs + cs, b, :].reshape(cs // P, P, D_MODEL)
                for b in range(B)
            ]
        ).reshape(n_it, P, D_MODEL)
        in_maps.append(
            {
                "hT": hT,
                "wq": np.ascontiguousarray(wq_cols.reshape(KCH, P, P)).astype(bfd),
                "wk": np.ascontiguousarray(wk_cols.reshape(KCH, P, P)).astype(bfd),
                "wv": np.ascontiguousarray(wv_cols.reshape(KCH, P, P)).astype(bfd),
                "wo": wo,
                "hres": np.ascontiguousarray(hres, dtype=np.float32),
                "lng": g,
                "lnb": bvec,
            }
        )
    return in_maps


def assemble_output(results, t=T_FULL, n_cores=N_CORES):
    cs = t // n_cores
    full = np.empty((t, B, D_MODEL), dtype=np.float32)
    for c in range(n_cores):
        o = results[c]["out"].reshape(B, cs, D_MODEL)
        for b in range(B):
            full[c * cs : (c + 1) * cs, b, :] = o[b]
    return full


def _numpy_fallback(h, attn_mask, Wq, Wkv, Wo, ln_g, ln_b):
    """Exact reference computation (only used if the mask is not causal)."""
    t, b, _ = h.shape
    hf = h.reshape(t * b, D_MODEL)
    q = (hf @ Wq).reshape(t, b, N_HEAD, D_HEAD)
    kv = (hf @ Wkv).reshape(t, b, N_HEAD, 2 * D_HEAD)
    k, v = kv[..., :D_HEAD], kv[..., D_HEAD:]
    s = np.einsum("ibnd,jbnd->ijbn", q, k) * SCALE
    s = np.where(attn_mask[:, :, :, None], -np.inf, s)
    s = s - s.max(axis=1, keepdims=True)
    p = np.exp(s)
    p = p / p.sum(axis=1, keepdims=True)
    av = np.einsum("ijbn,jbnd->ibnd", p, v).reshape(t, b, N_HEAD * D_HEAD)
    ao = av @ Wo
    x = h + ao
    mu = x.mean(axis=-1, keepdims=True)
    var = ((x - mu) ** 2).mean(axis=-1, keepdims=True)
    return ((x - mu) / np.sqrt(var + LN_EPS) * ln_g + ln_b).astype(np.float32)


_PROGRAM_CACHE = {}


def kernel(h, attn_mask, Wq, Wkv, Wo, ln_g, ln_b):
    global LAST_RESULT
    h = np.asarray(h, dtype=np.float32)
    attn_mask = np.asarray(attn_mask)
    Wq = np.asarray(Wq, dtype=np.float32)
    Wkv = np.asarray(Wkv, dtype=np.float32)
    Wo = np.asarray(Wo, dtype=np.float32)
    ln_g = np.asarray(ln_g, dtype=np.float32)
    ln_b = np.asarray(ln_b, dtype=np.float32)

    t = h.shape[0]
    causal = np.triu(np.ones((t, t), dtype=bool), k=1)
    if not np.array_equal(attn_mask, np.broadcast_to(causal[:, :, None], attn_mask.shape)):
        return _numpy_fallback(h, attn_mask, Wq, Wkv, Wo, ln_g, ln_b)

    apply_gb = not (np.all(ln_g == 1.0) and np.all(ln_b == 0.0))
    key = (t, apply_gb)
    if key not in _PROGRAM_CACHE:
        _PROGRAM_CACHE[key] = build_program(t=t, apply_gb=apply_gb)
    nc = _PROGRAM_CACHE[key]

    in_maps = make_in_maps(h, Wq, Wkv, Wo, ln_g, ln_b, t=t)
    res = run_bass_kernel_spmd(
        nc,
        in_maps,
        core_ids=list(range(N_CORES)),
        trace=bool(int(os.environ.get("KERNEL_TRACE", "0"))),
    )
    LAST_RESULT = res
    return assemble_output(res.results, t=t)


if __name__ == "__main__":
    build_program()
    print("program built ok")
